# revision 14
# baseline (speedup 1.0000x reference)
"""CVLoss Trainium2 kernel.

Computes the per-neuron coefficient-of-variation (CV) of inter-spike
intervals over a (B*T, N) spike train and the MSE loss against target CVs.

Sharding: neuron/model parallel — 8 cores x 128 neurons, each core gets its
contiguous (32768, 128) slice of the time-flattened train.

Per-core device pipeline (time processed in 2048-step chunks):
  - DMA loads the chunk time-major ([128 time, 16 blocks, 128 neurons]).
  - GPSIMD computes notm = 1 - m, downcast to fp16 (spike values are 0/1 so
    fp16 is exact).
  - PE transposes each 128x128 block to neuron-major PSUM (fp16), and also
    computes "nibble" matmuls: for every 4-timestep window,
    sum(notm * 2^(t%4)) — an exact invertible 4-bit mask of the window used
    on the host only to recover first-spike index and spike counts.
  - DVE runs the age recurrence a_t = (a_{t-1}+1)*(1-m_t) with the hardware
    scan instruction, reading notm^T straight from PSUM, chained across
    chunks via its carry.
  - ACT reduces sum(ages) per chunk (activation accumulate).

The ISI statistics then collapse to these sums by a telescoping identity:
    sum over spikes of gap^2 = 1 - (a_end+1)^2 + 2*sum(ages) + T
(including one blind first-spike gap (t_first+1)^2, removed on the host).
The final ~1024-neuron CV/MSE math runs on host in float32, replicating the
reference op-for-op.
"""

import numpy as np

import concourse.bacc as bacc
import concourse.bass as bass
import concourse.mybir as mybir
import concourse.tile as tile
from concourse import bass_utils

B, T_STEP, N = 16, 2048, 1024
TT = B * T_STEP              # 32768 total timesteps per neuron
NCORES = 8
NLOC = N // NCORES           # 128 neurons per core
CHUNK = 2048                 # main chunk size
# head chunks are small so the scan chain starts ~15us earlier
CHUNK_SIZES = [512] * 4 + [2048] * 15
assert sum(CHUNK_SIZES) == TT
NCHUNK = TT // CHUNK         # 16 (nibble-layout unit)
NBLK = CHUNK // 128          # 16 blocks per full chunk

F32 = mybir.dt.float32
F16 = mybir.dt.float16
AF = mybir.ActivationFunctionType
ALU = mybir.AluOpType
AX = mybir.AxisListType

# stats layout (columns of the [128, NSTAT] f32 output):
#   [0:48)     sum(ages) per half-chunk (variable count, zero-padded)
#   [48]       final a_end
SA0 = 0
AEND0 = 48
NSTAT = 49

# bitmask output: [128, NCHUNK*1024] f16; per chunk a [128, 1024] block laid
# out as partition p = 64*(blk%2) + c (c < 32 real, else zero), free =
# (blk//2)*128 + n, holding sum_{j<4} notm[t,n]*2^j for t = 128*blk+4*c+j.
BM_W = 1024


def _wmask_np():
    """[128, 64] fp16 nibble weights: W[t, c] = (t//4 == c) * 2^(t%4),
    columns 32..63 zero-padding (PE col-group alignment)."""
    w = np.zeros((128, 64), dtype=np.float16)
    for t in range(128):
        w[t, t // 4] = np.float16(2.0 ** (t % 4))
    return w


def build_kernel(tt=TT):
    nchunk = tt // CHUNK
    nc = bacc.Bacc("TRN2", target_bir_lowering=False, debug=False)
    spikes = nc.dram_tensor("spikes", [tt, NLOC], F32, kind="ExternalInput")
    ident = nc.dram_tensor("ident", [128, 128], F16, kind="ExternalInput")
    wmask = nc.dram_tensor("wmask", [128, 64], F16, kind="ExternalInput")
    stats = nc.dram_tensor("stats", [128, NSTAT], F32, kind="ExternalOutput")
    bmask = nc.dram_tensor("bmask", [128, NCHUNK * BM_W], F16, kind="ExternalOutput")

    sp = spikes.ap()

    with tile.TileContext(nc) as tc:
        with (
            tc.tile_pool(name="static", bufs=1) as static_pool,
            tc.tile_pool(name="raw", bufs=3) as raw_pool,
            tc.tile_pool(name="notm", bufs=2) as notm_pool,
            tc.tile_pool(name="ages", bufs=2) as ages_pool,
            tc.tile_pool(name="junk", bufs=1) as junk_pool,
            tc.tile_pool(name="bmsb", bufs=2) as bm_pool,
            tc.tile_pool(name="stats", bufs=1) as stats_pool,
            tc.tile_pool(name="psum", bufs=2, space="PSUM") as psum_pool,
            tc.tile_pool(name="psbm", bufs=2, space="PSUM") as psbm_pool,
            # PSUM budget: mt [128,1024]f32 = 2 banks x2 bufs + bm
            # [128,1024]f32 = 2 banks x2 bufs = 8 banks total
        ):
            ident_sb = static_pool.tile([128, 128], F16)
            nc.sync.dma_start(ident_sb[:], ident.ap())
            wmask_sb = static_pool.tile([128, 64], F16)
            nc.sync.dma_start(wmask_sb[:], wmask.ap())
            ones_sb = static_pool.tile([128, CHUNK // 2], F16)
            nc.gpsimd.memset(ones_sb[:], 1.0)

            statsb = stats_pool.tile([128, NSTAT], F32)
            nc.gpsimd.memset(statsb[:], 0.0)
            junk = junk_pool.tile([128, CHUNK // 2], F16)

            prev_ages = None
            prev_half = CHUNK // 2
            t0 = 0
            bm_off = 0
            n_sa = 0
            chunk_sizes = CHUNK_SIZES if tt == TT else [CHUNK] * nchunk
            for csize in chunk_sizes:
                nblk = csize // 128
                raw = raw_pool.tile([128, NBLK, 128], F32, tag="raw")
                nc.sync.dma_start(
                    raw[:, :nblk, :],
                    sp[t0:t0 + csize, :].rearrange("(a p) n -> p a n", p=128),
                )
                # notm = 1 - m, fp16 (GPSIMD, 1-input ~line-rate)
                notm = notm_pool.tile([128, NBLK, 128], F16, tag="notm")
                nc.gpsimd.tensor_scalar(
                    notm[:, :nblk, :], raw[:, :nblk, :], -1.0, 1.0,
                    ALU.mult, ALU.add
                )

                # PE nibble matmuls (batched per column-group, <=1 psum bank)
                bm = psbm_pool.tile([128, BM_W], F32, tag="bm")
                notm_qr = notm[:, :nblk, :].rearrange(
                    "p (q r) n -> p r q n", r=2
                )
                qtot = nblk // 2
                for r in range(2):
                    for qh in range(0, qtot, 4):
                        qn = min(4, qtot - qh)
                        nc.tensor.matmul(
                            bm[64 * r:64 * (r + 1),
                               qh * 128:(qh + qn) * 128],
                            wmask_sb[:],
                            notm_qr[:, r, qh:qh + qn],
                        )
                # bitmask evacuation PSUM -> SBUF (fp16 exact: values <= 15)
                bw = qtot * 128
                bmsb = bm_pool.tile([128, BM_W], F16, tag="bmsb")
                nc.scalar.copy(bmsb[:, :bw], bm[:, :bw])
                nc.sync.dma_start(
                    bmask.ap()[:, bm_off:bm_off + bw], bmsb[:, :bw]
                )
                bm_off += bw

                # transpose via regular matmul (notm^T = notm.T @ I), then
                # age scan per half chunk: state = (state + 1) * notm
                half = csize // 2
                for h in range(2):
                    mt = psum_pool.tile([128, CHUNK // 2], F32, tag="mt")
                    for b2 in range(half // 128):
                        blk = h * (half // 128) + b2
                        nc.tensor.matmul(
                            mt[:, b2 * 128:(b2 + 1) * 128],
                            notm[:, blk, :],
                            ident_sb[:],
                        )
                    ages = ages_pool.tile([128, CHUNK // 2], F16, tag="ages")
                    a_init = (
                        0.0 if prev_ages is None
                        else prev_ages[:, prev_half - 1:prev_half]
                    )
                    nc.vector.tensor_tensor_scan(
                        ages[:, :half], ones_sb[:, :half], mt[:, :half],
                        a_init, op0=ALU.add, op1=ALU.mult,
                    )
                    # sum(ages) via ACT accumulate (junk elementwise out)
                    nc.scalar.activation(
                        junk[:, :half], ages[:, :half], AF.Identity,
                        bias=0.0, scale=1.0,
                        accum_out=statsb[:, SA0 + n_sa:SA0 + n_sa + 1],
                    )
                    n_sa += 1
                    prev_ages = ages
                    prev_half = half
                t0 += csize
            # final a_end (age at T-1)
            nc.vector.tensor_copy(
                statsb[:, AEND0:AEND0 + 1],
                prev_ages[:, prev_half - 1:prev_half],
            )
            nc.sync.dma_start(stats.ap(), statsb[:])

    nc.compile()
    return nc


_CACHE = {}


def _get_nc():
    if "nc" not in _CACHE:
        _CACHE["nc"] = build_kernel()
    return _CACHE["nc"]


_POP = np.array([bin(i).count("1") for i in range(16)], dtype=np.int64)


def _chunk_schedule(tt=TT):
    return CHUNK_SIZES if tt == TT else [CHUNK] * (tt // CHUNK)


def _decode_bitmasks(bm, tt=TT):
    """bm: [128, >=tt/2] f16 of notm-nibbles -> (k, t_f) per neuron.

    Per chunk of size csize, a [128, csize/2] block: partition p = 64*r + c
    (c < 32 real window index), free = q*128 + n, blk = 2*q + r, covering
    t = t0 + 128*blk + 4*c + j with value sum(notm * 2^j); spike nibble is
    15 - value.
    """
    parts = []
    off = 0
    bmv = np.asarray(bm, dtype=np.float64)
    for csize in _chunk_schedule(tt):
        bw = csize // 2
        qtot = csize // 256
        v = np.round(bmv[:, off:off + bw]).astype(np.int64)
        v = v.reshape(2, 64, qtot, 128)          # [r, c, q, n]
        m_nib = (15 - v[:, :32]).transpose(3, 2, 0, 1)  # [n, q, r, c]
        parts.append(m_nib.reshape(128, qtot * 2 * 32))
        off += bw
    flat = np.concatenate(parts, axis=1)         # [n, tt/4] time-ordered
    k = _POP[flat].sum(axis=1)
    any_nib = flat > 0
    first_nib = np.argmax(any_nib, axis=1)
    has = any_nib.any(axis=1)
    nib_val = flat[np.arange(128), first_nib]
    low = np.zeros(128, dtype=np.int64)
    for j in range(3, -1, -1):
        low = np.where((nib_val >> j) & 1 == 1, j, low)
    t_f = np.where(has, first_nib * 4 + low, tt)
    return k.astype(np.float64), t_f.astype(np.float64)


def _finalize(stats_list, bmask_list, target_cv, tt=TT):
    """Combine per-core device stats into the scalar loss (host, float32)."""
    f32 = np.float32
    k_l, tf_l, tl_l, s2_l = [], [], [], []
    for st, bm in zip(stats_list, bmask_list):
        st = np.asarray(st, dtype=np.float64)
        nchunk = tt // CHUNK
        sum_a = st[:, SA0:AEND0].sum(axis=1)
        a_end = st[:, AEND0]
        k, t_f = _decode_bitmasks(bm, tt)
        t_l = tt - 1.0 - a_end              # == -1 when no spikes
        s2 = 1.0 - (a_end + 1.0) ** 2 + 2.0 * sum_a + tt - (t_f + 1.0) ** 2
        k_l.append(k); tf_l.append(t_f); tl_l.append(t_l); s2_l.append(s2)
    k = np.concatenate(k_l).astype(f32)
    t_f = np.concatenate(tf_l)
    t_l = np.concatenate(tl_l)
    s2 = np.concatenate(s2_l).astype(f32)
    tgt = np.asarray(target_cv, dtype=f32)

    n_isi = k - f32(1.0)
    sum_g = (t_l - t_f).astype(f32)
    mean = sum_g / np.maximum(n_isi, f32(1.0))
    var = (s2 - n_isi * mean * mean) / np.maximum(n_isi - f32(1.0), f32(1.0))
    std = np.sqrt(np.maximum(var, f32(0.0)).astype(f32))
    valid = (k >= f32(3.0)) & (mean > f32(0.0))
    cv = np.where(valid, std / np.where(mean > f32(0.0), mean, f32(1.0)), f32(0.0))
    sq = np.where(valid, (cv - tgt) ** 2, f32(0.0)).astype(f32)
    nvalid = valid.astype(f32).sum(dtype=f32)
    loss = np.where(
        nvalid > f32(0.0), sq.sum(dtype=f32) / np.maximum(nvalid, f32(1.0)), f32(0.0)
    )
    return np.asarray(loss, dtype=np.float32)


_IDENT = np.eye(128, dtype=np.float16)
_WMASK = _wmask_np()


def make_in_maps(output_spikes):
    s = np.asarray(output_spikes, dtype=np.float32).reshape(TT, N)
    return [
        {
            "spikes": np.ascontiguousarray(s[:, d * NLOC:(d + 1) * NLOC]),
            "ident": _IDENT,
            "wmask": _WMASK,
        }
        for d in range(NCORES)
    ]


def kernel(output_spikes, target_cv, _trace=False):
    nc = _get_nc()
    in_maps = make_in_maps(output_spikes)
    res = bass_utils.run_bass_kernel_spmd(
        nc, in_maps, core_ids=list(range(NCORES)), trace=_trace
    )
    _CACHE["last_result"] = res
    stats_list = [res.results[d]["stats"] for d in range(NCORES)]
    bmask_list = [res.results[d]["bmask"] for d in range(NCORES)]
    return _finalize(stats_list, bmask_list, target_cv)
